# revision 52
# baseline (speedup 1.0000x reference)
"""GIN message-passing kernel for 8 TRN2 NeuronCores.

Nodes are sharded across 8 cores (6272 slots each, 49 tiles of 128). Edges are
partitioned by destination tile; source rows are fetched per edge with
gpsimd.dma_gather from a replicated bf16 table (x for layer 1, AllGather'ed h1
for layer 2). The gather is DMA-round-trip bound per SWDGE queue context
(~9.9 ns/descriptor/queue), so the kernel's core trick is to spread gathers
across all 4 hardware SWDGE queues (num_swdge_queues=4, greedy per-gather
slot balancing) for ~4x gather throughput; everything else hides behind them:

- PAIR FETCHES: descriptor cost is size-independent up to 512B, so each
  descriptor fetches TWO adjacent bf16 table rows (one pair id, 512B). When
  both rows of a pair are needed by the same gather group, two fetches
  collapse into one. Pairs are not random: a greedy max-weight matching per
  (core, tile) puts nodes that co-occur as sources in the same (dst core,
  gather group) into adjacent slots, raising the collision rate well above
  birthday level. Each fetched pair-chunk yields two one-hot columns per
  covering tile (one per half).
- Gathers are fused per group of 2-7 tiles (GSIZES ramp: small first group
  shortens the pipeline prologue, small last groups the epilogue). Each
  (group, half) gather is split into two sub-gathers on independently
  chosen queues, holding worst-queue slot imbalance to <1%. A pair used by
  >= 2 distinct tiles of the group is fetched ONCE into the group's SHARED
  segment; every tile's one-hot covers the shared chunk range with its own
  destination offsets (P where unused). Remaining per-tile "singles" pack
  densely with spanning chunks (a chunk may straddle two tiles, masked per
  tile). All segments pad to the cross-core max with masked duplicates: one
  compile-time shape, no -1 sentinels, no G memset. Descriptors:
  74,496/layer/core for ~100k edges (was 106,624 before sharing/pairs).
- The one-hot M is built in a single DVE is_equal pass (offa vs iota).
  Duplicate-source pairing (a second offb mask) was removed: it saved ~1% of
  gather slots but tripled the DVE mask work, which sat on the critical path.
- Self-edges are NOT gathered: the (1+eps)*x_i term is added with one
  identity matmul per tile from a per-tile x_self DMA (layer 1) or from the
  SBUF-resident transposed h1 (layer 2).
- Aggregate and MLP stages are software-pipelined (depth 2) so the PE never
  stalls on the PSUM->SBUF agg copy.
- Table rows are laid out [band][core][tile][slot]; tiles 0-31 map to rows
  [0, 32768) (int16 gather-index limit) and tiles 32-48 to the hi band. The
  h1 AllGather is one Shared-output collective per band (bf16): AG_A is
  issued 2 gather-groups before layer-1 end (Pool is already throttled by
  the G-ring WAR wait there, so the collective's slice-store deps never
  stall gather descriptor generation); AG_B is issued after layer-2's first
  lo-gather, and layer-2 hi-gathers lag lo-gathers by LAG_G=2 groups to
  cover its transfer.
- Compute is bf16 end-to-end (tables, G, M, weights); PSUM stays fp32.
- single_packet=True reproducibly desyncs the mesh -- keep it False.

Queue-balanced DMA floor: 2 x 74,496 descs / 4 queues x 9.9 ns = 370 us.
Measured (quiet device): ~520-650 us vs 2152 us for the 1-queue baseline;
under ambient DMA contention from co-tenants both degrade toward ~1-2 ms.
"""
import warnings

warnings.filterwarnings("ignore")

import numpy as np

N = 50000
E = 800000
F = 128
H = 128
C = 40
BN_EPS = 1e-5
NCORES = 8
P = 128
NT = 49              # tiles per core
NPC = NT * P         # 6272 node slots per core
NPAD = NCORES * NPC  # 50176
TLO = 32             # tiles 0..31 -> low rows
ROWS_LO = NCORES * TLO * P        # 32768 == int16 gather limit
ROWS_HI = NPAD - ROWS_LO          # 17408

# Ramped gather-group sizes: small first group = short pipeline prologue
# (compute starts after ~1/2 the gather of a full group); small last groups =
# short epilogue; big middle groups maximize shared-segment dedup.
GSIZES = [3, 5, 7, 7, 7, 7, 7, 4, 2]
assert sum(GSIZES) == NT
NGRP = len(GSIZES)
_bnd = np.concatenate([[0], np.cumsum(GSIZES)])
GRPS = [list(range(int(_bnd[g]), int(_bnd[g + 1]))) for g in range(NGRP)]
GRP_OF = np.repeat(np.arange(NGRP), GSIZES)   # tile -> group
LAG_G = 2            # layer-2 hi-gather group lag (hides AG_B tail)
LO_AGC = [32]             # lo-band AllGather chunk sizes (tiles)
HI_AGC = [17]             # hi-band AllGather chunk sizes (tiles)


# ----------------------------------------------------------------- host prep

def _assign_nodes(deg):
    """Greedy balanced assignment of nodes to (core, tile, slot) by degree.

    Returns gid_of_orig[N]: gid = c*NPC + t*P + s.
    """
    order = np.argsort(-deg, kind="stable")
    core_load = np.zeros(NCORES, np.int64)
    core_cnt = np.zeros(NCORES, np.int64)
    node_core = np.empty(N, np.int32)
    for n in order:
        c = -1
        best = None
        for cc in range(NCORES):
            if core_cnt[cc] >= NPC:
                continue
            if best is None or core_load[cc] < best:
                best = core_load[cc]
                c = cc
        node_core[n] = c
        core_load[c] += deg[n]
        core_cnt[c] += 1

    gid_of_orig = np.empty(N, np.int64)
    for c in range(NCORES):
        nodes = order[node_core[order] == c]
        tile_load = np.zeros(NT, np.int64)
        tile_cnt = np.zeros(NT, np.int64)
        tl = np.empty(len(nodes), np.int32)
        for i, n in enumerate(nodes):
            avail = tile_cnt < P
            t = np.where(avail, tile_load, np.iinfo(np.int64).max).argmin()
            tl[i] = t
            tile_load[t] += deg[n]
            tile_cnt[t] += 1
        slot = np.zeros(NT, np.int64)
        for i, n in enumerate(nodes):
            t = tl[i]
            gid_of_orig[n] = c * NPC + t * P + slot[t]
            slot[t] += 1
    return gid_of_orig


def _assign_cells(node_ids, lo, hi, ncells, cap):
    """Greedy 2-D balance of nodes into ncells cells of capacity cap.

    Minimizes the per-cell max of normalized (lo, hi) loads. Returns cell id
    per node (aligned with node_ids order).
    """
    tot_lo, tot_hi = max(lo.sum(), 1), max(hi.sum(), 1)
    t_lo = tot_lo / ncells
    t_hi = tot_hi / ncells
    loads = np.zeros((ncells, 2), np.float64)
    cnt = np.zeros(ncells, np.int64)
    order = np.argsort(-(np.maximum(lo / t_lo, hi / t_hi)), kind="stable")
    cell_of = np.empty(len(node_ids), np.int64)
    for i in order:
        nl = (loads[:, 0] + lo[i]) / t_lo
        nh = (loads[:, 1] + hi[i]) / t_hi
        score = np.maximum(nl, nh)
        score[cnt >= cap] = np.inf
        cbest = int(np.argmin(score))
        cell_of[i] = cbest
        loads[cbest, 0] += lo[i]
        loads[cbest, 1] += hi[i]
        cnt[cbest] += 1
    return cell_of


def _band_chunk_meta():
    """Per-tile (chunk id, chunk size, tile offset in chunk) for both bands,
    plus per-chunk row bases (within the band)."""
    meta = {}
    lo_base = []
    b = 0
    for k, sz in enumerate(LO_AGC):
        lo_base.append(b)
        for j in range(sz):
            t = k * 8 + j if False else sum(LO_AGC[:k]) + j
            meta[t] = ("lo", k, sz, j, b)
        b += NCORES * sz * P
    hi_base = []
    b = 0
    start = 0
    for k, sz in enumerate(HI_AGC):
        hi_base.append(b)
        for j in range(sz):
            t = TLO + start + j
            meta[t] = ("hi", k, sz, j, b)
        b += NCORES * sz * P
        start += sz
    return meta, lo_base, hi_base


_CHUNK_META, _LO_BASE, _HI_BASE = _band_chunk_meta()


def _row_of_gid(gid):
    """Table row for gid with [band][AG chunk][core][tile][slot] layout.
    Lo rows in [0, ROWS_LO); hi rows returned as ROWS_LO + hi-band offset."""
    gid = np.asarray(gid, np.int64)
    c = gid // NPC
    r = gid % NPC
    t = r // P
    s = r % P
    band_base = np.empty(NT, np.int64)
    csize = np.empty(NT, np.int64)
    toff = np.empty(NT, np.int64)
    is_lo_t = np.zeros(NT, bool)
    for tt in range(NT):
        band, k, sz, j, b = _CHUNK_META[tt]
        band_base[tt] = b
        csize[tt] = sz
        toff[tt] = j
        is_lo_t[tt] = band == "lo"
    base = band_base[t] + c * (csize[t] * P) + toff[t] * P + s
    return np.where(is_lo_t[t], base, ROWS_LO + base)


def _wrap_idx(idx):
    """[n] int -> [128, n//16] int16: idx i at [i%16, i//16], replicated x8."""
    n = len(idx)
    w = np.asarray(idx, np.int16).reshape(n // 16, 16).T
    return np.tile(w, (8, 1))


def _pack_edges(src_row, dst_gid):
    """Partition edges by (core, tile), split lo/hi by source band, and pack
    each (core, group, half) as [per-tile singles | shared] over PAIR ids
    (pair = two adjacent table rows, fetched as one 512B descriptor):

    - An edge maps to (pair=row//2, h2=row%2, dst). A pair used by >= 2
      distinct tiles of the group goes to the SHARED segment; one slot can
      serve, per tile, one h2=0 edge AND one h2=1 edge.
    - Each fetched pair-chunk (128 pairs) yields TWO one-hot columns per
      covering tile (one per h2 half); offa columns are (chunk, half)-major.
    - All segments pad to the cross-core max with masked duplicates.

    Returns (key_a, key_b, idx_pack, offa_pack); k0/n keys count pair-chunks.
    """
    core = dst_gid // NPC
    tile = (dst_gid % NPC) // P
    off = dst_gid % P
    is_lo = src_row < ROWS_LO

    lists = [[None] * NT for _ in range(NCORES)]
    key = (core * NT + tile).astype(np.int64)
    order = np.argsort(key, kind="stable")
    rows_s, off_s, lo_s = src_row[order], off[order], is_lo[order]
    key_s = key[order]
    bounds = np.searchsorted(key_s, np.arange(NCORES * NT + 1))
    for c in range(NCORES):
        for t in range(NT):
            b0, b1 = bounds[c * NT + t], bounds[c * NT + t + 1]
            m = lo_s[b0:b1]
            rl, ol = rows_s[b0:b1][m], off_s[b0:b1][m]
            rh, oh = rows_s[b0:b1][~m] - ROWS_LO, off_s[b0:b1][~m]
            lists[c][t] = ((rl // 2, rl % 2, ol), (rh // 2, rh % 2, oh))

    # split each (core, group, half) into per-tile singles + shared slots.
    # singles[c][t][half] = (pairs, off2[n,2]); shared[c][g][half] =
    # (pairs, offmat[n, ngrp, 2]) with P where unused.
    singles = [[[None, None] for _ in range(NT)] for _ in range(NCORES)]
    shared = [[[None, None] for _ in range(NGRP)] for _ in range(NCORES)]
    for c in range(NCORES):
        for g, grp in enumerate(GRPS):
            ng = len(grp)
            for half in (0, 1):
                pr_all = np.concatenate([lists[c][t][half][0] for t in grp])
                h2_all = np.concatenate([lists[c][t][half][1] for t in grp])
                of_all = np.concatenate(
                    [lists[c][t][half][2] for t in grp]).astype(np.float32)
                tis_all = np.concatenate(
                    [np.full(len(lists[c][t][half][0]), i, np.int64)
                     for i, t in enumerate(grp)])
                if len(pr_all) == 0:
                    for i, t in enumerate(grp):
                        singles[c][t][half] = (
                            np.zeros(0, np.int64), np.zeros((0, 2), np.float32))
                    shared[c][g][half] = (
                        np.zeros(0, np.int64),
                        np.zeros((0, ng, 2), np.float32))
                    continue
                o = np.argsort(pr_all, kind="stable")
                r, h2, ti, of = pr_all[o], h2_all[o], tis_all[o], of_all[o]
                grp_start = np.r_[True, r[1:] != r[:-1]]
                gidx = np.cumsum(grp_start) - 1
                rt_start = np.r_[True,
                                 (r[1:] != r[:-1]) | (ti[1:] != ti[:-1])]
                ntile = np.bincount(gidx[rt_start], minlength=gidx[-1] + 1)
                multi = ntile[gidx] >= 2

                def _slots(rr, hh, oo):
                    """pair slots for edges of ONE tile: per pair, slot j
                    holds j-th h2=0 edge and j-th h2=1 edge."""
                    if len(rr) == 0:
                        return (np.zeros(0, np.int64),
                                np.zeros((0, 2), np.float32))
                    ps, o2 = [], []
                    us = np.flatnonzero(np.r_[True, rr[1:] != rr[:-1]])
                    ue = np.r_[us[1:], len(rr)]
                    for s0, s1 in zip(us, ue):
                        e0 = [oo[j] for j in range(s0, s1) if hh[j] == 0]
                        e1 = [oo[j] for j in range(s0, s1) if hh[j] == 1]
                        for jj in range(max(len(e0), len(e1))):
                            ps.append(int(rr[s0]))
                            o2.append((e0[jj] if jj < len(e0) else P,
                                       e1[jj] if jj < len(e1) else P))
                    return (np.asarray(ps, np.int64),
                            np.asarray(o2, np.float32).reshape(len(ps), 2))

                for i, t in enumerate(grp):
                    m = (~multi) & (ti == i)
                    singles[c][t][half] = _slots(r[m], h2[m], of[m])
                rm = r[multi]
                him, tim, ofm = h2[multi], ti[multi], of[multi]
                sh_rows, sh_mat = [], []
                if len(rm):
                    us = np.flatnonzero(np.r_[True, rm[1:] != rm[:-1]])
                    ue = np.r_[us[1:], len(rm)]
                    for s0, s1 in zip(us, ue):
                        per = {}
                        for j in range(s0, s1):
                            per.setdefault(
                                (int(tim[j]), int(him[j])), []).append(ofm[j])
                        copies = max(len(v) for v in per.values())
                        for jj in range(copies):
                            colv = np.full((ng, 2), P, np.float32)
                            for (tii, hh), v in per.items():
                                if jj < len(v):
                                    colv[tii, hh] = v[jj]
                            sh_rows.append(int(rm[s0]))
                            sh_mat.append(colv)
                shared[c][g][half] = (
                    np.asarray(sh_rows, np.int64),
                    np.asarray(sh_mat, np.float32).reshape(
                        len(sh_rows), ng, 2))

    MAX_LO = np.array([max(len(singles[c][t][0][0]) for c in range(NCORES))
                       for t in range(NT)], np.int64)
    MAX_HI = np.array([max(len(singles[c][t][1][0]) for c in range(NCORES))
                       for t in range(NT)], np.int64)
    MAXSH_LO = np.array([max(len(shared[c][g][0][0]) for c in range(NCORES))
                         for g in range(NGRP)], np.int64)
    MAXSH_HI = np.array([max(len(shared[c][g][1][0]) for c in range(NCORES))
                         for g in range(NGRP)], np.int64)

    def _padded(c, t, half, nslots):
        pairs, o2 = singles[c][t][half]
        nreal = len(pairs)
        li = np.empty(nslots, np.int64)
        la = np.full((nslots, 2), P, np.float32)
        li[:nreal] = pairs
        la[:nreal] = o2
        pad = nslots - nreal
        if pad > 0:
            li[nreal:] = (np.resize(pairs, pad) if nreal
                          else (np.arange(pad, dtype=np.int64) * 97) % 1024)
        return li, la

    def _padded_sh(c, g, half, nslots, ng):
        pairs, mat = shared[c][g][half]
        nreal = len(pairs)
        li = np.empty(nslots, np.int64)
        lm = np.full((nslots, ng, 2), P, np.float32)
        li[:nreal] = pairs
        lm[:nreal] = mat
        pad = nslots - nreal
        if pad > 0:
            li[nreal:] = (np.resize(pairs, pad) if nreal
                          else (np.arange(pad, dtype=np.int64) * 97) % 1024)
        return li, lm

    CHL_G = np.zeros(NGRP, np.int64)
    CHH_G = np.zeros(NGRP, np.int64)
    start_lo = np.zeros(NT, np.int64)
    start_hi = np.zeros(NT, np.int64)
    startsh_lo = np.zeros(NGRP, np.int64)
    startsh_hi = np.zeros(NGRP, np.int64)
    for g, grp in enumerate(GRPS):
        acc = 0
        for t in grp:
            start_lo[t] = acc
            acc += int(MAX_LO[t])
        startsh_lo[g] = acc
        acc += int(MAXSH_LO[g])
        CHL_G[g] = (acc + P - 1) // P
        acc = 0
        for t in grp:
            start_hi[t] = acc
            acc += int(MAX_HI[t])
        startsh_hi[g] = acc
        acc += int(MAXSH_HI[g])
        CHH_G[g] = (acc + P - 1) // P
    k0_lo = start_lo // P
    n_lo = (start_lo + MAX_LO + P - 1) // P - k0_lo
    k0_hi = start_hi // P
    n_hi = (start_hi + MAX_HI + P - 1) // P - k0_hi
    k0sh_lo = startsh_lo // P
    nsh_lo = (startsh_lo + MAXSH_LO + P - 1) // P - k0sh_lo
    k0sh_hi = startsh_hi // P
    nsh_hi = (startsh_hi + MAXSH_HI + P - 1) // P - k0sh_hi

    def _cols2(n, k0, st, nsl, vals2):
        """[P, 2n] offa cols, (chunk, half)-major, vals2 [nsl, 2] at st."""
        arr = np.full((n * P, 2), P, np.float32)
        s0 = int(st) - int(k0) * P
        arr[s0:s0 + nsl] = vals2
        return arr.reshape(n, P, 2).transpose(1, 0, 2).reshape(P, 2 * n)

    idx_pack, offa_pack = [], []
    for c in range(NCORES):
        idx_cols, offa_cols = [], []
        for g, grp in enumerate(GRPS):
            for half, CHG, MAXC, MAXSH in (
                (0, CHL_G, MAX_LO, MAXSH_LO), (1, CHH_G, MAX_HI, MAXSH_HI),
            ):
                nslots = int(CHG[g]) * P
                li = np.empty(nslots, np.int64)
                li[:] = (np.arange(nslots, dtype=np.int64) * 97) % 1024
                acc = 0
                for t in grp:
                    lt, _ = _padded(c, t, half, int(MAXC[t]))
                    li[acc:acc + int(MAXC[t])] = lt
                    acc += int(MAXC[t])
                lsh, _ = _padded_sh(c, g, half, int(MAXSH[g]), len(grp))
                li[acc:acc + int(MAXSH[g])] = lsh
                idx_cols.append(_wrap_idx(li))
        for t in range(NT):
            g = int(GRP_OF[t])
            ti = t - int(_bnd[g])
            for (half, MAXC, st, k0, nn, MAXSH, stsh, k0sh, nshn) in (
                (0, MAX_LO, start_lo, k0_lo, n_lo,
                 MAXSH_LO, startsh_lo, k0sh_lo, nsh_lo),
                (1, MAX_HI, start_hi, k0_hi, n_hi,
                 MAXSH_HI, startsh_hi, k0sh_hi, nsh_hi),
            ):
                n = int(nn[t])
                if n > 0:
                    _, la = _padded(c, t, half, int(MAXC[t]))
                    offa_cols.append(_cols2(n, k0[t], st[t], int(MAXC[t]), la))
                nsh = int(nshn[g])
                if nsh > 0:
                    _, lm = _padded_sh(c, g, half, int(MAXSH[g]), len(GRPS[g]))
                    offa_cols.append(_cols2(nsh, k0sh[g], stsh[g],
                                            int(MAXSH[g]), lm[:, ti, :]))
        idx_pack.append(np.ascontiguousarray(np.concatenate(idx_cols, axis=1)))
        offa_pack.append(np.ascontiguousarray(np.concatenate(offa_cols, axis=1)))
    key_a = np.concatenate([CHL_G, k0_lo, n_lo, k0sh_lo, nsh_lo])
    key_b = np.concatenate([CHH_G, k0_hi, n_hi, k0sh_hi, nsh_hi])
    return key_a, key_b, idx_pack, offa_pack


def _bf16(a):
    import ml_dtypes
    return np.ascontiguousarray(np.asarray(a, np.float32).astype(
        ml_dtypes.bfloat16))


def prepare(x, edge_index, W1a, bn_gamma, bn_beta, bn_mean, bn_var, W1b, W2a, W2b):
    x = np.asarray(x, np.float32)
    ei = np.asarray(edge_index, np.int64)
    src_o, dst_o = ei[0], ei[1]

    deg = np.bincount(dst_o, minlength=N).astype(np.int64)
    gid1 = _assign_nodes(deg)                      # phase 1: defines bands
    band_lo = ((gid1 % NPC) // P) < TLO            # per node, padded later
    # per-node lo/hi in-degree (by source band) -- stable under phase 2
    src_lo = band_lo[src_o]
    lo_in = np.bincount(dst_o[src_lo], minlength=N).astype(np.int64)
    hi_in = deg - lo_in

    # phase 2: rebalance within each band across all (core, tile) cells
    gid_of_orig = np.empty(N, np.int64)
    for in_band, tset in ((band_lo, np.arange(TLO)),
                          (~band_lo, np.arange(TLO, NT))):
        nodes = np.flatnonzero(in_band)
        ncells = NCORES * len(tset)
        cell_of = _assign_cells(nodes, lo_in[nodes].astype(np.float64),
                                hi_in[nodes].astype(np.float64), ncells, P)
        for k in np.unique(cell_of):
            members = nodes[cell_of == k]
            c = k // len(tset)
            t = tset[k % len(tset)]
            base = c * NPC + t * P
            gid_of_orig[members] = base + np.arange(len(members))

    # Pair-matching pass: reorder slots within each (core, tile) so nodes
    # that co-occur as sources in the same (dst core, gather group) sit in
    # adjacent slots -- each shared context turns two 256B fetches into one
    # 512B pair fetch. Greedy max-weight matching on the co-occurrence
    # counts; any slot permutation is valid for the packing machinery.
    NCELL = NCORES * NGRP
    dcell = ((gid_of_orig[dst_o] // NPC) * NGRP
             + GRP_OF[(gid_of_orig[dst_o] % NPC) // P])
    inc = np.zeros((N, NCELL), np.int8)
    inc[src_o, dcell] = 1
    gid_order = np.argsort(gid_of_orig, kind="stable")
    gid_vals = gid_of_orig[gid_order]
    for c in range(NCORES):
        for t in range(NT):
            base = c * NPC + t * P
            b0 = np.searchsorted(gid_vals, base)
            b1 = np.searchsorted(gid_vals, base + P)
            members = gid_order[b0:b1]     # node ids in this tile (<= 128)
            nm = len(members)
            if nm < 2:
                continue
            Im = inc[members].astype(np.float32)
            Cm = Im @ Im.T
            np.fill_diagonal(Cm, -1.0)
            flat = np.argsort(Cm, axis=None)[::-1]
            used = np.zeros(nm, bool)
            order = []
            for f in flat:
                u, v = divmod(int(f), nm)
                if u >= v or used[u] or used[v]:
                    continue
                used[u] = used[v] = True
                order.append(u)
                order.append(v)
                if len(order) >= nm - 1:
                    break
            if len(order) < nm:
                order.extend(np.flatnonzero(~used).tolist())
            gid_of_orig[members[np.asarray(order)]] = base + np.arange(nm)
    row_of_gid = _row_of_gid(np.arange(NPAD, dtype=np.int64))

    src_row = row_of_gid[gid_of_orig[src_o]]
    dst_gid = gid_of_orig[dst_o]
    key_a, key_b, idx_pack, offa_pack = _pack_edges(src_row, dst_gid)

    # x table in row order (bf16)
    x_pad = np.zeros((NPAD, F), np.float32)
    x_pad[row_of_gid[gid_of_orig]] = x

    # per-core self rows in (tile, slot) order
    x_gid = np.zeros((NPAD, F), np.float32)
    x_gid[gid_of_orig] = x

    scale = (np.asarray(bn_gamma) / np.sqrt(np.asarray(bn_var) + BN_EPS)
             ).astype(np.float32)
    bias = (np.asarray(bn_beta) - np.asarray(bn_mean) * scale).astype(
        np.float32)

    consts = {
        "x_pad": _bf16(x_pad).reshape(NPAD // 2, 2 * F),
        "W1aT": _bf16(np.asarray(W1a, np.float32).T),
        "W1bT": _bf16(np.asarray(W1b, np.float32).T),
        "W2aT": _bf16(np.asarray(W2a, np.float32).T),
        "W2bT": _bf16(np.asarray(W2b, np.float32).T),
        "bn_s": scale.reshape(H, 1),
        "bn_b": bias.reshape(H, 1),
        "iota": _bf16(np.tile(np.arange(P, dtype=np.float32), (P, 1))),
    }
    in_maps = []
    for c in range(NCORES):
        m = dict(consts)
        m["idx_all"] = idx_pack[c]
        m["offa_all"] = _bf16(offa_pack[c])
        m["x_self"] = _bf16(x_gid[c * NPC:(c + 1) * NPC])
        in_maps.append(m)
    return in_maps, key_a, key_b, gid_of_orig


# -------------------------------------------------------------- bass program

def build(key_a, key_b, do_gather=True, do_compute=True, do_cc=True,
          nqueues=4):
    import concourse.bacc as bacc
    import concourse.mybir as mybir
    import concourse.tile as tile
    from concourse.masks import make_identity

    key_a = np.asarray(key_a, np.int64)
    key_b = np.asarray(key_b, np.int64)
    CHL_G = [int(v) for v in key_a[:NGRP]]
    k0_lo = key_a[NGRP:NGRP + NT]
    n_lo = key_a[NGRP + NT:NGRP + 2 * NT]
    k0sh_lo = key_a[NGRP + 2 * NT:2 * NGRP + 2 * NT]
    nsh_lo = key_a[2 * NGRP + 2 * NT:]
    CHH_G = [int(v) for v in key_b[:NGRP]]
    k0_hi = key_b[NGRP:NGRP + NT]
    n_hi = key_b[NGRP + NT:NGRP + 2 * NT]
    k0sh_hi = key_b[NGRP + 2 * NT:2 * NGRP + 2 * NT]
    nsh_hi = key_b[2 * NGRP + 2 * NT:]

    nc = bacc.Bacc("TRN2", target_bir_lowering=False, debug=False,
                   num_devices=NCORES, num_swdge_queues=nqueues)
    f32 = mybir.dt.float32
    bf16 = mybir.dt.bfloat16

    CHG_G = [l + h for l, h in zip(CHL_G, CHH_G)]
    CHG_MAX = max(CHG_G)
    S_TOT = int(8 * sum(CHG_G))
    gcol = np.concatenate([[0], np.cumsum([c * 8 for c in CHG_G])])
    # offa column base per tile: [lo singles|lo shared|hi singles|hi shared]
    gofs = GRP_OF.astype(np.int64)
    percol = 2 * (n_lo + nsh_lo[gofs] + n_hi + nsh_hi[gofs])
    ocol = np.concatenate([[0], np.cumsum(percol)])
    CH_TOT = int(ocol[-1])

    x_pad = nc.dram_tensor("x_pad", [NPAD // 2, 2 * F], bf16,
                          kind="ExternalInput")
    x_self = nc.dram_tensor("x_self", [NPC, F], bf16, kind="ExternalInput")
    idx_all = nc.dram_tensor("idx_all", [P, S_TOT], mybir.dt.int16,
                             kind="ExternalInput")
    offa_all = nc.dram_tensor("offa_all", [P, CH_TOT], bf16, kind="ExternalInput")
    W1aT = nc.dram_tensor("W1aT", [F, H], bf16, kind="ExternalInput")
    W1bT = nc.dram_tensor("W1bT", [H, H], bf16, kind="ExternalInput")
    W2aT = nc.dram_tensor("W2aT", [H, H], bf16, kind="ExternalInput")
    W2bT = nc.dram_tensor("W2bT", [H, C], bf16, kind="ExternalInput")
    bn_s = nc.dram_tensor("bn_s", [H, 1], f32, kind="ExternalInput")
    bn_b = nc.dram_tensor("bn_b", [H, 1], f32, kind="ExternalInput")
    iota = nc.dram_tensor("iota", [P, P], bf16, kind="ExternalInput")
    outT = nc.dram_tensor("outT", [C, NPC], f32, kind="ExternalOutput")

    Relu = mybir.ActivationFunctionType.Relu
    Copy = mybir.ActivationFunctionType.Copy

    with tile.TileContext(nc) as tc:
        with (
            tc.tile_pool(name="const", bufs=1) as cst,
            tc.tile_pool(name="gbuf", bufs=3) as gp,
            tc.tile_pool(name="mbuf", bufs=2) as mp,
            tc.tile_pool(name="small", bufs=6) as sp,
            tc.tile_pool(name="ps_agg", bufs=2, space="PSUM") as ps_agg,
            tc.tile_pool(name="ps_t", bufs=2, space="PSUM") as ps_t,
            tc.tile_pool(name="ps_mm", bufs=2, space="PSUM") as ps_mm,
            tc.tile_pool(name="dram", bufs=1, space="DRAM") as dram,
        ):
            ident = cst.tile([P, P], f32)
            make_identity(nc, ident[:])
            identb = cst.tile([P, P], bf16)
            nc.scalar.activation(out=identb[:], in_=ident[:], func=Copy)
            iota_sb = cst.tile([P, P], bf16)
            nc.sync.dma_start(out=iota_sb[:], in_=iota[:])
            w1a_sb = cst.tile([F, H], bf16)
            nc.sync.dma_start(out=w1a_sb[:], in_=W1aT[:])
            w1b_sb = cst.tile([H, H], bf16)
            nc.sync.dma_start(out=w1b_sb[:], in_=W1bT[:])
            w2a_sb = cst.tile([H, H], bf16)
            nc.sync.dma_start(out=w2a_sb[:], in_=W2aT[:])
            w2b_sb = cst.tile([H, C], bf16)
            nc.sync.dma_start(out=w2b_sb[:], in_=W2bT[:])
            bns_sb = cst.tile([H, 1], f32)
            nc.sync.dma_start(out=bns_sb[:], in_=bn_s[:])
            bnb_sb = cst.tile([H, 1], f32)
            nc.sync.dma_start(out=bnb_sb[:], in_=bn_b[:])
            idx_sb = cst.tile([P, S_TOT], mybir.dt.int16)
            nc.sync.dma_start(out=idx_sb[:], in_=idx_all[:])
            offa_sb = cst.tile([P, CH_TOT], bf16)
            nc.sync.dma_start(out=offa_sb[:], in_=offa_all[:])
            h1keep = cst.tile([P, NT * P], bf16)   # transposed h1, bf16
            if not do_gather:
                G_shared = cst.tile([P, CHG_MAX, 2 * F], bf16)
                nc.vector.memset(G_shared[:], 0.0)

            slice_a = dram.tile([TLO * P, H], bf16)
            slice_b = dram.tile([(NT - TLO) * P, H], bf16)
            full_a = dram.tile([ROWS_LO // 2, 2 * H], bf16,
                               addr_space="Shared")
            full_b = dram.tile([ROWS_HI // 2, 2 * H], bf16,
                               addr_space="Shared")

            # Greedy queue balancing by descriptor-slot count.
            qload = np.zeros(nqueues, np.int64)

            def _pick_queue(slots):
                q = int(np.argmin(qload))
                qload[q] += slots
                return q

            def _gather_split(tab, G, gbase, ic, chn):
                # split into two sub-gathers on independently chosen queues:
                # halves the worst-queue slot imbalance and smooths temporal
                # interleaving of the 4 DMA queue contexts
                for c0, c1 in ((0, (chn + 1) // 2), ((chn + 1) // 2, chn)):
                    w = c1 - c0
                    if w <= 0:
                        continue
                    nc.gpsimd.dma_gather(
                        G[:, gbase + c0:gbase + c1, :], tab,
                        idx_sb[:, ic + c0 * 8:ic + c1 * 8],
                        w * P, w * P, 2 * F, single_packet=False,
                        queue_num=_pick_queue(w))

            def gather_lo_grp(g, tab_lo, G):
                chl = CHL_G[g]
                if chl == 0 or not do_gather:
                    return
                _gather_split(tab_lo, G, 0, int(gcol[g]), chl)

            def gather_hi_grp(g, tab_hi, G):
                chl, chh = CHL_G[g], CHH_G[g]
                if chh == 0 or not do_gather:
                    return
                _gather_split(tab_hi, G, chl, int(gcol[g]) + chl * 8, chh)

            def aggregate(g, t, G, self_sb):
                """one-hot segment-sum of G's tile-t chunks plus self term."""
                nl, nh = int(n_lo[t]), int(n_hi[t])
                nsl, nsh = int(nsh_lo[g]), int(nsh_hi[g])
                chp = nl + nsl + nh + nsh     # pair chunks
                ch = 2 * chp                  # logical one-hot columns
                M = mp.tile([P, ch * P], bf16, tag="M", name="M")
                oc = int(ocol[t])
                nc.vector.tensor_tensor(
                    out=M[:, :ch * P],
                    in0=offa_sb[:, oc:oc + ch, None].to_broadcast([P, ch, P]),
                    in1=iota_sb[:, None, :].to_broadcast([P, ch, P]),
                    op=mybir.AluOpType.is_equal,
                )
                agg_ps = ps_agg.tile([F, P], f32, tag="agg", name="agg_ps")
                CHL = CHG_MAX if not do_gather else CHL_G[g]
                for j in range(ch):
                    k, hf = j // 2, j % 2
                    if not do_gather:
                        gk = k % CHG_MAX
                    elif k < nl:
                        gk = int(k0_lo[t]) + k
                    elif k < nl + nsl:
                        gk = int(k0sh_lo[g]) + (k - nl)
                    elif k < nl + nsl + nh:
                        gk = CHL + int(k0_hi[t]) + (k - nl - nsl)
                    else:
                        gk = CHL + int(k0sh_hi[g]) + (k - nl - nsl - nh)
                    nc.tensor.matmul(out=agg_ps[:],
                                     lhsT=G[:, gk, hf * F:(hf + 1) * F],
                                     rhs=M[:, j * P:(j + 1) * P],
                                     start=(j == 0), stop=False)
                nc.tensor.matmul(out=agg_ps[:], lhsT=self_sb, rhs=identb[:],
                                 start=False, stop=True)
                agg_sb = sp.tile([F, P], bf16, tag="agg_sb", name="agg_sb")
                nc.scalar.activation(out=agg_sb[:], in_=agg_ps[:], func=Copy)
                return agg_sb

            # lo AG chunk k is emitted at the start of layer-1 group
            # iteration g(last tile of chunk) + 3 -- by then the Pool engine
            # is already throttled by the G-ring WAR wait, so the AG's
            # slice-store deps are satisfied and desc-gen never stalls.
            ag_lo_at = {}
            tend = 0
            for k, sz in enumerate(LO_AGC):
                tend += sz
                g_emit = min(int(GRP_OF[tend - 1]) + 2, NGRP - 1)
                ag_lo_at.setdefault(g_emit, []).append(k)

            def issue_ag(band, k):
                if not (do_cc and do_compute):
                    return
                if band == "lo":
                    sz = LO_AGC[k]
                    t0 = sum(LO_AGC[:k])
                    ins = slice_a[t0 * P:(t0 + sz) * P, :]
                    base = _LO_BASE[k]
                    outs = full_a[base // 2:(base + NCORES * sz * P) // 2, :]
                else:
                    sz = HI_AGC[k]
                    t0 = sum(HI_AGC[:k])
                    ins = slice_b[t0 * P:(t0 + sz) * P, :]
                    base = _HI_BASE[k]
                    outs = full_b[base // 2:(base + NCORES * sz * P) // 2, :]
                nc.gpsimd.collective_compute(
                    "AllGather", mybir.AluOpType.bypass,
                    replica_groups=[list(range(NCORES))],
                    ins=[ins.opt()], outs=[outs.opt()],
                )

            # ---------------- layer 1 ----------------
            def l1_mlp(t, agg_sb):
                h1a_ps = ps_mm.tile([H, P], f32, tag="mma", name="h1a_ps")
                nc.tensor.matmul(out=h1a_ps[:], lhsT=w1a_sb[:], rhs=agg_sb[:],
                                 start=True, stop=True)
                h1a_sb = sp.tile([H, P], bf16, tag="h1a", name="h1a_sb")
                nc.scalar.activation(out=h1a_sb[:], in_=h1a_ps[:], func=Relu,
                                     bias=bnb_sb[:, :1], scale=bns_sb[:, :1])
                h1b_ps = ps_mm.tile([H, P], f32, tag="mmb", name="h1b_ps")
                nc.tensor.matmul(out=h1b_ps[:], lhsT=w1b_sb[:], rhs=h1a_sb[:],
                                 start=True, stop=True)
                h1b_sb = sp.tile([H, P], f32, tag="h1b", name="h1b_sb")
                nc.scalar.activation(out=h1b_sb[:], in_=h1b_ps[:], func=Relu)
                ht_ps = ps_t.tile([P, H], f32, tag="trans", name="ht_ps")
                nc.tensor.transpose(out=ht_ps[:], in_=h1b_sb[:],
                                    identity=ident[:])
                ht_sb = sp.tile([P, H], bf16, tag="ht", name="ht_sb")
                nc.scalar.activation(out=ht_sb[:], in_=ht_ps[:], func=Copy)
                nc.vector.tensor_copy(out=h1keep[:, t * P:(t + 1) * P],
                                      in_=ht_ps[:])
                if t < TLO:
                    nc.sync.dma_start(
                        out=slice_a[t * P:(t + 1) * P, :], in_=ht_sb[:])
                else:
                    tt = t - TLO
                    nc.sync.dma_start(
                        out=slice_b[tt * P:(tt + 1) * P, :], in_=ht_sb[:])

            pend = []

            def drain(n, mlp):
                while len(pend) > n:
                    mlp(*pend.pop(0))

            for g, grp in enumerate(GRPS):
                for k in ag_lo_at.get(g, []):
                    issue_ag("lo", k)
                G = (gp.tile([P, CHG_G[g], 2 * F], bf16, tag="G", name="G")
                     if do_gather else G_shared)
                gather_lo_grp(g, x_pad[0:ROWS_LO // 2, :], G)
                gather_hi_grp(g, x_pad[ROWS_LO // 2:NPAD // 2, :], G)
                if do_compute:
                    for t in grp:
                        xs = sp.tile([P, F], bf16, tag="xs", name="xs")
                        nc.sync.dma_start(
                            out=xs[:], in_=x_self[t * P:(t + 1) * P, :])
                        pend.append((t, aggregate(g, t, G, xs[:])))
                        drain(1, l1_mlp)
            if do_compute:
                drain(0, l1_mlp)

            # ---------------- layer 2 ----------------
            def l2_mlp(t, agg_sb):
                h2_ps = ps_mm.tile([H, P], f32, tag="mma", name="h2_ps")
                nc.tensor.matmul(out=h2_ps[:], lhsT=w2a_sb[:], rhs=agg_sb[:],
                                 start=True, stop=True)
                h2_sb = sp.tile([H, P], bf16, tag="h1a", name="h2_sb")
                nc.scalar.activation(out=h2_sb[:], in_=h2_ps[:], func=Relu)
                o_ps = ps_mm.tile([C, P], f32, tag="mmb", name="o_ps")
                nc.tensor.matmul(out=o_ps[:], lhsT=w2b_sb[:], rhs=h2_sb[:],
                                 start=True, stop=True)
                o_sb = sp.tile([C, P], f32, tag="out", name="o_sb")
                nc.scalar.activation(out=o_sb[:], in_=o_ps[:], func=Relu)
                nc.sync.dma_start(out=outT[:, t * P:(t + 1) * P], in_=o_sb[:])

            glist = [None] * NGRP
            for g in range(NGRP + LAG_G):
                if g < NGRP:
                    G = (gp.tile([P, CHG_G[g], 2 * F], bf16, tag="G",
                                 name="G")
                         if do_gather else G_shared)
                    glist[g] = G
                    gather_lo_grp(g, full_a[:], G)
                if g < len(HI_AGC):
                    # hi AG chunk g: slice stores long done; transfer overlaps
                    # the LAG_G groups of lo gathers ahead of the hi gathers.
                    issue_ag("hi", g)
                if g >= LAG_G:
                    gh = g - LAG_G
                    gather_hi_grp(gh, full_b[:], glist[gh])
                    if do_compute:
                        for t in GRPS[gh]:
                            pend.append((t, aggregate(
                                gh, t, glist[gh],
                                h1keep[:, t * P:(t + 1) * P])))
                            drain(1, l2_mlp)
            if do_compute:
                drain(0, l2_mlp)

    nc.compile()
    return nc


# ------------------------------------------------------------------- driver

_CACHE = {}


def kernel(x, edge_index, W1a, bn_gamma, bn_beta, bn_mean, bn_var, W1b, W2a, W2b,
           _trace=False):
    from concourse.bass_utils import run_bass_kernel_spmd

    in_maps, key_a, key_b, gid_of_orig = prepare(
        x, edge_index, W1a, bn_gamma, bn_beta, bn_mean, bn_var, W1b, W2a, W2b
    )
    key = (tuple(key_a), tuple(key_b))
    if key not in _CACHE:
        _CACHE[key] = build(key_a, key_b)
    nc = _CACHE[key]

    res = run_bass_kernel_spmd(nc, in_maps, core_ids=list(range(NCORES)))
    outT = np.concatenate([r["outT"] for r in res.results], axis=1)  # [C, NPAD]
    out = outT.T[gid_of_orig]  # [N, C]
    if _trace:
        kernel.last_results = res
    return np.ascontiguousarray(out.astype(np.float32))
